# revision 1
# baseline (speedup 1.0000x reference)
"""NativeSparseAttention (fallback = full causal SDPA) Trainium2 kernel.

Sharding: 8 cores = 2 (batch) x 4 (kv head groups). Core (b, g) computes
q heads 4g..4g+3, kv head g, batch b, and a row-parallel partial of the
output projection; partials are summed on the host (the "all-reduce").

Layouts on device (per core):
  xT    [1024, 2048] bf16   hidden_states[b].T
  qT    [256, 2048]  bf16   feature-major q (RoPE applied), Wq pre-scaled 1/8
  kT    [64, 2048]   bf16   feature-major k (RoPE applied)
  v     [2048, 65]   bf16   token-major v with ones column (softmax denom)
  pT    [kv, sq]            scores transposed; exp on ACT; causal diag mask
  attn  [sq, 65]     f32    PSUM accumulated over kv chunks; col 64 = denom
  ag    [2048, 256]  bf16   gated/normalized attn, token-major
  agT   [256, 2048]  bf16   PE-transposed for output projection
  outp  [2048, 1024] f32    partial output
"""

import numpy as np
import ml_dtypes

import concourse.bass as bass
import concourse.mybir as mybir
import concourse.tile as tile
from concourse.bass_utils import run_bass_kernel_spmd
from concourse.masks import make_identity

FP32 = mybir.dt.float32
BF16 = mybir.dt.bfloat16
AF = mybir.ActivationFunctionType
ALU = mybir.AluOpType


def _patch_tail_drain():
    """This container's walrus build allows only ONE semaphore wait per CTRL
    (Drain/NoOp) instruction, but Tile's kernel-tail drain attaches one wait
    per active queue/engine. Split the waits across preceding single-wait
    NOPs on the same engine (SP executes them in order, so semantics are
    unchanged)."""
    from bass_rust import ScopedClock

    if getattr(tile.TileContext, "_tail_drain_patched", False):
        return

    def _drain_and_barrier(self, tick_clock, wait_clock):
        nc = self.nc
        probe = nc.sync.nop(nofuse=True)
        wait_clock.add_sem_waits(
            probe.ins, ScopedClock({None: tick_clock.global_clock})
        )
        si = probe.ins.sync_info
        waits = list(si.on_wait) if si is not None else []
        if len(waits) > 1:
            si.on_wait = waits[:1]
            for w in waits[1:]:
                n2 = nc.sync.nop(nofuse=True)
                n2.ins.sync_info = mybir.SyncInfo(on_wait=[w], on_update=[])
        nc.sync.drain()
        nc.all_engine_barrier()
        popped = nc._tile_sem_poison_stack.pop()
        assert popped is self._sem_poison
        nc.clear_and_free_semaphores(list(self.sems.allocated().values()))
        nc.all_engine_barrier()

    tile.TileContext._drain_and_barrier = _drain_and_barrier
    tile.TileContext._tail_drain_patched = True


_patch_tail_drain()

B = 2
S = 2048
HM = 1024
NH = 16
NKV = 4
D = 64
THETA = 10000.0
NCORES = 8

NCH = S // 128  # 16 sequence chunks of 128


def _split_multi_waits(nc: bass.Bass):
    """Walrus here allows a single semaphore wait per instruction; hoist
    extra waits onto same-engine NOPs placed immediately before (same
    sequencer, in-order => identical semantics)."""
    for f in nc.m.functions:
        for b in f.blocks:
            new = []
            changed = False
            for ins in b.instructions:
                si = ins.sync_info
                waits = list(si.on_wait) if si is not None else []
                if len(waits) > 1:
                    changed = True
                    for i, w in enumerate(waits[:-1]):
                        nop = mybir.InstNoOp(
                            name=f"{ins.name}-sw{i}",
                            sync_info=mybir.SyncInfo(on_wait=[w], on_update=[]),
                            bass_nofuse=True,
                            engine=ins.engine,
                        )
                        nc.register_instruction(nop, overwrite=True)
                        new.append(nop)
                    si.on_wait = waits[-1:]
                new.append(ins)
            if changed:
                b.instructions = new



def _build_program() -> bass.Bass:
    nc = bass.Bass(trn_type="TRN2", target_bir_lowering=False, debug=False)

    xT = nc.dram_tensor("xT", [HM, S], BF16, kind="ExternalInput").ap()
    # weights pre-interleaved on host: [128, hm_chunk * width] so each loads
    # in ONE contiguous DMA (24 small serialized DMAs cost ~15us otherwise)
    wqT = nc.dram_tensor("wqT", [128, 8 * 256], BF16, kind="ExternalInput").ap()
    wkT = nc.dram_tensor("wkT", [128, 8 * 64], BF16, kind="ExternalInput").ap()
    wvgT = nc.dram_tensor("wvgT", [128, 8 * 72], BF16, kind="ExternalInput").ap()
    woT = nc.dram_tensor("woT", [256, HM], BF16, kind="ExternalInput").ap()
    cosT = nc.dram_tensor("cosT", [128, S], BF16, kind="ExternalInput").ap()
    sinT = nc.dram_tensor("sinT", [128, S], BF16, kind="ExternalInput").ap()
    dmask = nc.dram_tensor("dmask", [128, 128], BF16, kind="ExternalInput").ap()
    outp = nc.dram_tensor("outp", [S, HM], FP32, kind="ExternalOutput").ap()

    with tile.TileContext(nc) as tc:
        with (
            tc.tile_pool(name="const", bufs=1) as cpool,
            tc.tile_pool(name="acts", bufs=1) as apool,
        ):
            # ---- constant / weight loads (small weights first so the q/k/v
            # projections can start as soon as the first x chunk lands) ----
            wvg_all = cpool.tile([128, 8 * 72], BF16, tag="wvg")
            nc.sync.dma_start(wvg_all[:], wvgT[:, :])
            wk_all = cpool.tile([128, 8 * 64], BF16, tag="wk")
            nc.sync.dma_start(wk_all[:], wkT[:, :])
            wq_all = cpool.tile([128, 8 * 256], BF16, tag="wq")
            nc.sync.dma_start(wq_all[:], wqT[:, :])
            wvg_sb = [wvg_all[:, i * 72 : (i + 1) * 72] for i in range(8)]
            wk_sb = [wk_all[:, i * 64 : (i + 1) * 64] for i in range(8)]
            wq_sb = [wq_all[:, i * 256 : (i + 1) * 256] for i in range(8)]
            x_sb = []
            for i in range(8):
                t = cpool.tile([128, S], BF16, tag=f"x{i}", name=f"x{i}")
                nc.sync.dma_start(t[:], xT[i * 128 : (i + 1) * 128, :])
                x_sb.append(t)
            cos_sb = cpool.tile([128, S], BF16, tag="cos")
            nc.sync.dma_start(cos_sb[:], cosT[:, :])
            sin_sb = cpool.tile([128, S], BF16, tag="sin")
            nc.sync.dma_start(sin_sb[:], sinT[:, :])
            dmask_sb = cpool.tile([128, 128], BF16, tag="dmask")
            nc.sync.dma_start(dmask_sb[:], dmask[:, :])
            wo_sb = []
            for j in range(2):
                t = cpool.tile([128, HM], BF16, tag=f"wo{j}", name=f"wo{j}")
                nc.sync.dma_start(t[:], woT[j * 128 : (j + 1) * 128, :])
                wo_sb.append(t)
            ident_sb = cpool.tile([128, 128], BF16, tag="ident")
            make_identity(nc, ident_sb[:])

            # ---- persistent activations ----
            qT_sb = [apool.tile([64, S], BF16, tag=f"qT{h}", name=f"qT{h}") for h in range(4)]
            kT_sb = apool.tile([64, S], BF16, tag="kT")
            v_sb = [apool.tile([128, 65], BF16, tag=f"v{s}", name=f"v{s}") for s in range(NCH)]
            g_sb = [apool.tile([128, 4], FP32, tag=f"g{s}", name=f"g{s}") for s in range(NCH)]
            ag_sb = [apool.tile([128, 256], BF16, tag=f"ag{s}", name=f"ag{s}") for s in range(NCH)]
            agT_sb = [
                [
                    apool.tile([128, 128], BF16, tag=f"agT{j}_{s}", name=f"agT{j}_{s}")
                    for s in range(NCH)
                ]
                for j in range(2)
            ]

            # ---- unified compute scope ----
            # PSUM budget (8 banks): st 4 + acc 2 + mix 2. "mix" is shared by
            # the projection psum tiles, the v+gates psum tiles and the
            # head-3 transpose/out-projection tiles (disjoint lifetimes).
            with (
                tc.tile_pool(name="st", bufs=2, space="PSUM") as stpool,
                tc.tile_pool(name="acc", bufs=2, space="PSUM") as accpool,
                tc.tile_pool(name="mix", bufs=2, space="PSUM") as mixpool,
                tc.tile_pool(name="pt", bufs=34) as ptpool,
                tc.tile_pool(name="rl", bufs=8) as rlpool,
                tc.tile_pool(name="ost", bufs=3) as ostpool,
                tc.tile_pool(name="rope", bufs=4) as rpool,
                tc.tile_pool(name="gtmp", bufs=4) as gpool,
            ):
                def rope(ps, nsl, parts):
                    """RoPE a feature-major psum tile ps [parts, 512] in
                    bf16; returns (m1, rb) bf16 tiles whose sum is the
                    rotated q/k. rb holds the UNSIGNED half-rotation (rows
                    [0:32] <- src[32:64] and vice versa); the rotation sign
                    is folded into the host sin table. ACT does the
                    psum->bf16 conversion; DVE runs in its fast bf16 modes.
                    """
                    qb = rpool.tile([parts, 512], BF16, tag="qb", name="qb")
                    nc.scalar.copy(qb[:], ps[:parts, :])
                    rb = rpool.tile([parts, 512], BF16, tag="rb", name="rb")
                    m1 = rpool.tile([parts, 512], BF16, tag="m1", name="m1")
                    for h0 in range(0, parts, 64):
                        nc.vector.tensor_copy(
                            rb[h0 : h0 + 32, :], qb[h0 + 32 : h0 + 64, :]
                        )
                        nc.vector.tensor_copy(
                            rb[h0 + 32 : h0 + 64, :], qb[h0 : h0 + 32, :]
                        )
                    nc.vector.tensor_tensor(
                        m1[:], qb[:], cos_sb[:parts, nsl], op=ALU.mult
                    )
                    nc.vector.tensor_tensor(
                        rb[:], rb[:], sin_sb[:parts, nsl], op=ALU.mult
                    )
                    return m1, rb

                def emit_k_proj():
                    for n in range(4):
                        nsl = bass.ts(n, 512)
                        ps = mixpool.tile([64, 512], FP32, tag="mix", name="psk")
                        for kk in range(8):
                            nc.tensor.matmul(
                                ps[:],
                                wk_sb[kk][:],
                                x_sb[kk][:, nsl],
                                start=(kk == 0),
                                stop=(kk == 7),
                            )
                        m1, rb = rope(ps, nsl, 64)
                        nc.vector.tensor_tensor(
                            kT_sb[:, nsl], m1[:], rb[:], op=ALU.add
                        )

                def emit_q_proj(m):
                    for n in range(4):
                        nsl = bass.ts(n, 512)
                        ps = mixpool.tile([128, 512], FP32, tag="mix", name="psq")
                        for kk in range(8):
                            nc.tensor.matmul(
                                ps[:],
                                wq_sb[kk][:, m * 128 : (m + 1) * 128],
                                x_sb[kk][:, nsl],
                                start=(kk == 0),
                                stop=(kk == 7),
                            )
                        m1, rb = rope(ps, nsl, 128)
                        nc.vector.tensor_tensor(
                            qT_sb[2 * m][:, nsl], m1[0:64, :], rb[0:64, :],
                            op=ALU.add,
                        )
                        nc.vector.tensor_tensor(
                            qT_sb[2 * m + 1][:, nsl],
                            m1[64:128, :],
                            rb[64:128, :],
                            op=ALU.add,
                        )

                def emit_vg():
                    # v + gates, token-major; 4 sq-chunks share one psum tile
                    # (each vg result is only 72 cols) so the 2-slot mix pool
                    # rotation doesn't serialize 16 tiny tiles.
                    for s0 in range(0, NCH, 4):
                        ps = mixpool.tile([128, 288], FP32, tag="mix", name="psvg")
                        for sub in range(4):
                            s = s0 + sub
                            ssl = bass.ts(s, 128)
                            for kk in range(8):
                                nc.tensor.matmul(
                                    ps[:, sub * 72 : (sub + 1) * 72],
                                    x_sb[kk][:, ssl],
                                    wvg_sb[kk][:],
                                    start=(sub == 0 and kk == 0),
                                    stop=(sub == 3 and kk == 7),
                                )
                        for sub in range(4):
                            s = s0 + sub
                            o = sub * 72
                            nc.vector.tensor_copy(
                                v_sb[s][:, 0:64], ps[:, o : o + 64]
                            )
                            nc.vector.memset(v_sb[s][:, 64:65], 1.0)
                            # gate: G = 1 + 0.5*(tanh(a/2) + tanh(b/2))
                            tg = gpool.tile([128, 8], FP32, tag="tg", name="tg")
                            nc.scalar.activation(
                                tg[:], ps[:, o + 64 : o + 72], AF.Tanh, scale=0.5
                            )
                            gs = gpool.tile([128, 4], FP32, tag="gs", name="gs")
                            nc.gpsimd.tensor_tensor(
                                gs[:], tg[:, 0:4], tg[:, 4:8], op=ALU.add
                            )
                            nc.gpsimd.tensor_scalar(
                                g_sb[s][:], gs[:], 0.5, 1.0,
                                op0=ALU.mult, op1=ALU.add,
                            )

                def emit_scores(h, c):
                    """scores.T [kv 128, sq width] -> exp'd bf16 pt tiles
                    (one per 1024-col range)."""
                    qh = qT_sb[h]
                    width = S - c * 128
                    pts = []
                    for t0 in range(0, width, 1024):
                        cols = min(1024, width - t0)
                        st = stpool.tile([128, 1024], FP32, tag="st", name="st")
                        pt = ptpool.tile([128, 1024], BF16, tag="pt", name="pt")
                        pts.append(pt)
                        for n0 in range(0, cols, 512):
                            nn = min(512, cols - n0)
                            nc.tensor.matmul(
                                st[:, n0 : n0 + nn],
                                kT_sb[:, c * 128 : (c + 1) * 128],
                                qh[:, c * 128 + t0 + n0 : c * 128 + t0 + n0 + nn],
                                start=True,
                                stop=True,
                            )
                        nc.scalar.activation(pt[:, 0:cols], st[:, 0:cols], AF.Exp)
                        if t0 == 0:
                            # causal mask on the diagonal chunk (Pool:
                            # SBUF-only, keeps DVE free)
                            nc.gpsimd.tensor_tensor(
                                pt[:, 0:128], pt[:, 0:128], dmask_sb[:],
                                op=ALU.mult,
                            )
                    return pts

                def emit_sq(h, s, pts_by_c):
                    """P@V over kv chunks for one sq chunk (col 64 = softmax
                    denominator), then the gating epilogue; on head 3 also
                    transpose + output projection + DMA."""
                    acc = accpool.tile([128, 65], FP32, tag="acc", name="acc")
                    for c in range(s + 1):
                        off = (s - c) * 128
                        nc.tensor.matmul(
                            acc[:],
                            pts_by_c[c][off // 1024][:, off % 1024 : off % 1024 + 128],
                            v_sb[c][:],
                            start=(c == 0),
                            stop=(c == s),
                        )
                    rl = rlpool.tile([128, 1], FP32, tag="rl", name="rl")
                    nc.vector.reciprocal(rl[:], acc[:, 64:65])
                    nc.vector.tensor_scalar(
                        ag_sb[s][:, h * 64 : (h + 1) * 64],
                        acc[:, 0:64],
                        rl[:],
                        g_sb[s][:, h : h + 1],
                        op0=ALU.mult,
                        op1=ALU.mult,
                    )
                    if h == 3:
                        for j in range(2):
                            tp = mixpool.tile([128, 128], BF16, tag="mix", name="tp")
                            nc.tensor.transpose(
                                tp[:],
                                ag_sb[s][:, j * 128 : (j + 1) * 128],
                                ident_sb[:],
                            )
                            # ACT still runs head-3 exps for early chunks;
                            # late chunks split copies evenly DVE/ACT
                            if s >= 10 and j == 1:
                                nc.scalar.copy(agT_sb[j][s][:], tp[:])
                            else:
                                nc.vector.tensor_copy(agT_sb[j][s][:], tp[:])
                        ost = ostpool.tile([128, HM], FP32, tag="ost", name="ost")
                        for n in range(2):
                            po = mixpool.tile([128, 512], FP32, tag="mix", name="po")
                            for j in range(2):
                                nc.tensor.matmul(
                                    po[:],
                                    agT_sb[j][s][:],
                                    wo_sb[j][:, n * 512 : (n + 1) * 512],
                                    start=(j == 0),
                                    stop=(j == 1),
                                )
                            if s >= 10 and n == 1:
                                nc.scalar.copy(
                                    ost[:, n * 512 : (n + 1) * 512], po[:]
                                )
                            else:
                                nc.vector.tensor_copy(
                                    ost[:, n * 512 : (n + 1) * 512], po[:]
                                )
                        nc.sync.dma_start(outp[s * 128 : (s + 1) * 128, :], ost[:])

                # Emission order: k and q(m0) projections, then head-0 scores
                # (exp work reaches ACT ~20us earlier), then v+gates and
                # q(m1), then head-0 sq work, then heads 1-3 with scores
                # streaming 2 chunks ahead of sq work (PE runs in program
                # order, so each P@V's exp must already be emitted well
                # before it).
                emit_k_proj()
                emit_q_proj(0)
                h0_pts = [emit_scores(0, c) for c in range(NCH)]
                emit_vg()
                emit_q_proj(1)
                for s in range(NCH):
                    emit_sq(0, s, h0_pts)
                for h in range(1, 4):
                    pts_by_c = []
                    for c in range(NCH):
                        pts_by_c.append(emit_scores(h, c))
                        if c >= 2:
                            emit_sq(h, c - 2, pts_by_c)
                    emit_sq(h, NCH - 2, pts_by_c)
                    emit_sq(h, NCH - 1, pts_by_c)

    _split_multi_waits(nc)
    return nc


_NC = None


def _get_nc() -> bass.Bass:
    global _NC
    if _NC is None:
        _NC = _build_program()
    return _NC


def _shard_inputs(
    hidden_states, Wq, Wk, Wv, Wo, Wkc, Wg_slc, Wg_swa
) -> list[dict[str, np.ndarray]]:
    bf16 = ml_dtypes.bfloat16
    f32 = np.float32

    # RoPE tables (bf16, feature-major, duplicated across two 64-row head
    # blocks). The device computes the UNSIGNED half-rotation, so the
    # rotation sign is folded in here: sinP[d] = -sin for d<32, +sin for
    # d>=32.
    inv = 1.0 / (THETA ** (np.arange(0, D, 2, dtype=np.float64) / D))
    freqs = np.arange(S, dtype=np.float64)[:, None] * inv  # [S, 32]
    emb = np.concatenate([freqs, freqs], axis=-1)  # [S, 64]
    cosT = np.cos(emb).T  # [64, S]
    sinT = np.sin(emb).T
    sinT = np.concatenate([-sinT[0:32], sinT[32:64]], axis=0)
    cos2 = np.concatenate([cosT, cosT], axis=0).astype(bf16)  # [128, S]
    sin2 = np.concatenate([sinT, sinT], axis=0).astype(bf16)

    # pt[kv_i, sq_j] is valid iff kv <= sq, i.e. i <= j: upper triangular
    dmask = np.triu(np.ones((128, 128), dtype=f32)).astype(bf16)

    def interleave(w):
        """[1024, width] -> [128, 8*width] with hm-chunk-major columns so
        the whole weight loads in one contiguous DMA."""
        width = w.shape[1]
        return np.ascontiguousarray(
            w.reshape(8, 128, width).transpose(1, 0, 2).reshape(128, 8 * width)
        )

    in_maps = []
    for core in range(NCORES):
        b, g = divmod(core, 4)
        xTc = np.ascontiguousarray(hidden_states[b].T).astype(bf16)
        wqTc = interleave(
            np.ascontiguousarray((Wq[g * 256 : (g + 1) * 256, :] / 8.0).T).astype(
                bf16
            )
        )
        wkTc = interleave(
            np.ascontiguousarray(Wk[g * 64 : (g + 1) * 64, :].T).astype(bf16)
        )
        wvg = np.concatenate(
            [
                Wv[g * 64 : (g + 1) * 64, :].T,
                Wg_slc[g * 4 : (g + 1) * 4, :].T,
                Wg_swa[g * 4 : (g + 1) * 4, :].T,
            ],
            axis=1,
        )  # [1024, 72]
        wvgc = interleave(np.ascontiguousarray(wvg).astype(bf16))
        woTc = np.ascontiguousarray(Wo[:, g * 256 : (g + 1) * 256].T).astype(bf16)
        in_maps.append(
            {
                "xT": xTc,
                "wqT": wqTc,
                "wkT": wkTc,
                "wvgT": wvgc,
                "woT": woTc,
                "cosT": cos2,
                "sinT": sin2,
                "dmask": dmask,
            }
        )
    return in_maps


def run(inputs: dict, trace: bool = False):
    """Run the SPMD kernel; returns (output [B,S,HM] f32, BassKernelResults)."""
    nc = _get_nc()
    in_maps = _shard_inputs(**inputs)
    res = run_bass_kernel_spmd(
        nc, in_maps, core_ids=list(range(NCORES)), trace=trace
    )
    out = np.zeros((B, S, HM), np.float32)
    for core in range(NCORES):
        b = core // 4
        out[b] += res.results[core]["outp"]
    return out, res


def kernel(**inputs) -> np.ndarray:
    out, _ = run(inputs)
    return out



# revision 12
# speedup vs baseline: 1.0645x; 1.0645x over previous
"""NativeSparseAttention (fallback = full causal SDPA) Trainium2 kernel.

Sharding: 8 cores = 2 (batch) x 4 (kv head groups). Core (b, g) computes
q heads 4g..4g+3, kv head g, batch b, and a row-parallel partial of the
output projection; partials are summed on the host (the "all-reduce").

Schedule (v2): x/cos/sin stream in REVERSE column-block order (n=3..0) so
projections + head-0 scores start ~3us in; heads 1-3 stream scores 2
chunks ahead of P@V; head-3 sq carries the output projection; output is
fp16 partials. exp bottlenecks ACT (~58us floor), so everything else is
spread across PE/DVE/Pool.

Layouts on device (per core):
  xT    [1024, 2048] bf16   hidden_states[b].T (4 col-block DMAs per tile)
  qT    [256, 2048]  bf16   feature-major q (RoPE applied)
  kT    [64, 2048]   bf16   feature-major k (RoPE applied)
  v     [2048, 65]   bf16   token-major v with ones column (softmax denom)
  pT    [kv, sq]            scores transposed; exp(st/8) on ACT; diag mask
  attn  [sq, 65]     f32    PSUM accumulated over kv chunks; col 64 = denom
  ag    [2048, 256]  bf16   gated/normalized attn, token-major
  agT   [256, 2048]  bf16   PE-transposed for output projection
  outp  [2048, 1024] fp16   partial output
"""

import numpy as np
import ml_dtypes

import concourse.bass as bass
import concourse.mybir as mybir
import concourse.tile as tile
from concourse.bass_utils import run_bass_kernel_spmd
from concourse.masks import make_identity

FP32 = mybir.dt.float32
FP16 = mybir.dt.float16
BF16 = mybir.dt.bfloat16
AF = mybir.ActivationFunctionType
ALU = mybir.AluOpType


def _patch_tail_drain():
    """This container's walrus build allows only ONE semaphore wait per CTRL
    (Drain/NoOp) instruction, but Tile's kernel-tail drain attaches one wait
    per active queue/engine. Split the waits across preceding single-wait
    NOPs on the same engine (SP executes them in order, so semantics are
    unchanged)."""
    from bass_rust import ScopedClock

    if getattr(tile.TileContext, "_tail_drain_patched", False):
        return

    def _drain_and_barrier(self, tick_clock, wait_clock):
        nc = self.nc
        probe = nc.sync.nop(nofuse=True)
        wait_clock.add_sem_waits(
            probe.ins, ScopedClock({None: tick_clock.global_clock})
        )
        si = probe.ins.sync_info
        waits = list(si.on_wait) if si is not None else []
        if len(waits) > 1:
            si.on_wait = waits[:1]
            for w in waits[1:]:
                n2 = nc.sync.nop(nofuse=True)
                n2.ins.sync_info = mybir.SyncInfo(on_wait=[w], on_update=[])
        nc.sync.drain()
        nc.all_engine_barrier()
        popped = nc._tile_sem_poison_stack.pop()
        assert popped is self._sem_poison
        nc.clear_and_free_semaphores(list(self.sems.allocated().values()))
        nc.all_engine_barrier()

    tile.TileContext._drain_and_barrier = _drain_and_barrier
    tile.TileContext._tail_drain_patched = True


_patch_tail_drain()

B = 2
S = 2048
HM = 1024
NH = 16
NKV = 4
D = 64
THETA = 10000.0
NCORES = 8

NCH = S // 128  # 16 sequence chunks of 128


def _split_multi_waits(nc: bass.Bass):
    """Walrus here allows a single semaphore wait per instruction; hoist
    extra waits onto same-engine NOPs placed immediately before (same
    sequencer, in-order => identical semantics)."""
    for f in nc.m.functions:
        for b in f.blocks:
            new = []
            changed = False
            for ins in b.instructions:
                si = ins.sync_info
                waits = list(si.on_wait) if si is not None else []
                if len(waits) > 1:
                    changed = True
                    for i, w in enumerate(waits[:-1]):
                        nop = mybir.InstNoOp(
                            name=f"{ins.name}-sw{i}",
                            sync_info=mybir.SyncInfo(on_wait=[w], on_update=[]),
                            bass_nofuse=True,
                            engine=ins.engine,
                        )
                        nc.register_instruction(nop, overwrite=True)
                        new.append(nop)
                    si.on_wait = waits[-1:]
                new.append(ins)
            if changed:
                b.instructions = new



def _build_program() -> bass.Bass:
    nc = bass.Bass(trn_type="TRN2", target_bir_lowering=False, debug=False)

    # x is host-interleaved to [128, nblock, kk, 512] so each 512-col
    # n-block loads in ONE DMA; cos|sin are packed per n-block the same way.
    xTi = nc.dram_tensor("xTi", [128, 4 * 8 * 512], BF16, kind="ExternalInput").ap()
    wqT = nc.dram_tensor("wqT", [128, 8 * 256], BF16, kind="ExternalInput").ap()
    wkT = nc.dram_tensor("wkT", [128, 8 * 64], BF16, kind="ExternalInput").ap()
    wvgT = nc.dram_tensor("wvgT", [128, 8 * 72], BF16, kind="ExternalInput").ap()
    woT = nc.dram_tensor("woT", [256, HM], BF16, kind="ExternalInput").ap()
    csT = nc.dram_tensor("csT", [128, 4 * 1024], BF16, kind="ExternalInput").ap()
    dmask = nc.dram_tensor("dmask", [128, 128], BF16, kind="ExternalInput").ap()
    outp = nc.dram_tensor("outp", [S, HM], FP16, kind="ExternalOutput").ap()

    with tile.TileContext(nc) as tc:
        with (
            tc.tile_pool(name="const", bufs=1) as cpool,
            tc.tile_pool(name="acts", bufs=1) as apool,
        ):
            # ---- persistent tiles; DMAs are emitted inside the n-loop so
            # issue order matches need (reverse n) ----
            wq_all = cpool.tile([128, 8 * 256], BF16, tag="wq")
            wk_all = cpool.tile([128, 8 * 64], BF16, tag="wk")
            wvg_all = cpool.tile([128, 8 * 72], BF16, tag="wvg")
            wvg_sb = [wvg_all[:, i * 72 : (i + 1) * 72] for i in range(8)]
            wk_sb = [wk_all[:, i * 64 : (i + 1) * 64] for i in range(8)]
            wq_sb = [wq_all[:, i * 256 : (i + 1) * 256] for i in range(8)]
            x_all = cpool.tile([128, 8 * S], BF16, tag="xall")
            x_sb = [x_all[:, i * S : (i + 1) * S] for i in range(8)]
            cs_all = cpool.tile([128, 4 * 1024], BF16, tag="cs")
            cos_n = [cs_all[:, n * 1024 : n * 1024 + 512] for n in range(4)]
            sin_n = [cs_all[:, n * 1024 + 512 : n * 1024 + 1024] for n in range(4)]
            dmask_sb = cpool.tile([128, 128], BF16, tag="dmask")
            wo_sb = [
                cpool.tile([128, HM], BF16, tag=f"wo{j}", name=f"wo{j}")
                for j in range(2)
            ]
            ident_sb = cpool.tile([128, 128], BF16, tag="ident")

            qT_sb = [
                apool.tile([64, S], BF16, tag=f"qT{h}", name=f"qT{h}")
                for h in range(4)
            ]
            kT_sb = apool.tile([64, S], BF16, tag="kT")
            v_sb = [
                apool.tile([128, 65], BF16, tag=f"v{s}", name=f"v{s}")
                for s in range(NCH)
            ]
            g_sb = [
                apool.tile([128, 4], FP32, tag=f"g{s}", name=f"g{s}")
                for s in range(NCH)
            ]
            ag_sb = [
                apool.tile([128, 256], BF16, tag=f"ag{s}", name=f"ag{s}")
                for s in range(NCH)
            ]
            agT_sb = [
                [
                    apool.tile([128, 128], BF16, tag=f"agT{j}_{s}", name=f"agT{j}_{s}")
                    for s in range(NCH)
                ]
                for j in range(2)
            ]

            # PSUM budget (8 banks): st 4 + acc 2 + mix 2. "mix" is shared by
            # the projection psum tiles, the v+gates psum tiles and the
            # transpose/out-projection tiles (disjoint lifetimes).
            with (
                tc.tile_pool(name="st", bufs=2, space="PSUM") as stpool,
                tc.tile_pool(name="acc", bufs=2, space="PSUM") as accpool,
                tc.tile_pool(name="mix", bufs=2, space="PSUM") as mixpool,
                tc.tile_pool(name="pt", bufs=48) as ptpool,
                tc.tile_pool(name="rl", bufs=8) as rlpool,
                tc.tile_pool(name="ost", bufs=3) as ostpool,
                tc.tile_pool(name="rope", bufs=4) as rpool,
                tc.tile_pool(name="gtmp", bufs=8) as gpool,
            ):
                def rope(ps, n, parts, qb_on_act):
                    """RoPE a feature-major psum tile ps [parts, 512];
                    returns (m1, rb) bf16 tiles whose sum is the rotated
                    q/k. rb = shifted(qb) * sinP with the rotation sign
                    folded into the host sin table; the shift is fused into
                    the multiply's input AP (input bases must match, so the
                    sin table is stored half-swapped on the host)."""
                    cosv, sinv = cos_n[n], sin_n[n]
                    qb = rpool.tile([parts, 512], BF16, tag="qb", name="qb")
                    if qb_on_act:
                        nc.scalar.copy(qb[:], ps[:parts, :])
                    else:
                        nc.vector.tensor_copy(qb[:], ps[:parts, :])
                    rb = rpool.tile([parts, 512], BF16, tag="rb", name="rb")
                    m1 = rpool.tile([parts, 512], BF16, tag="m1", name="m1")
                    for h0 in range(0, parts, 64):
                        nc.vector.tensor_tensor(
                            rb[h0 : h0 + 32, :],
                            qb[h0 + 32 : h0 + 64, :],
                            sinv[h0 + 32 : h0 + 64, :],
                            op=ALU.mult,
                        )
                        nc.vector.tensor_tensor(
                            rb[h0 + 32 : h0 + 64, :],
                            qb[h0 : h0 + 32, :],
                            sinv[h0 : h0 + 32, :],
                            op=ALU.mult,
                        )
                    nc.vector.tensor_tensor(
                        m1[:], qb[:], cosv[:parts, :], op=ALU.mult
                    )
                    return m1, rb

                def emit_k_proj(n):
                    nsl = bass.ts(n, 512)
                    ps = mixpool.tile([64, 512], FP32, tag="mix", name="psk")
                    for kk in range(8):
                        nc.tensor.matmul(
                            ps[:],
                            wk_sb[kk][:],
                            x_sb[kk][:, nsl],
                            start=(kk == 0),
                            stop=(kk == 7),
                        )
                    m1, rb = rope(ps, n, 64, qb_on_act=True)
                    nc.vector.tensor_tensor(
                        kT_sb[:, nsl], m1[:], rb[:], op=ALU.add
                    )

                def emit_q_proj(m, n, qb_on_act):
                    nsl = bass.ts(n, 512)
                    ps = mixpool.tile([128, 512], FP32, tag="mix", name="psq")
                    for kk in range(8):
                        nc.tensor.matmul(
                            ps[:],
                            wq_sb[kk][:, m * 128 : (m + 1) * 128],
                            x_sb[kk][:, nsl],
                            start=(kk == 0),
                            stop=(kk == 7),
                        )
                    m1, rb = rope(ps, n, 128, qb_on_act=qb_on_act)
                    nc.vector.tensor_tensor(
                        qT_sb[2 * m][:, nsl], m1[0:64, :], rb[0:64, :],
                        op=ALU.add,
                    )
                    nc.vector.tensor_tensor(
                        qT_sb[2 * m + 1][:, nsl],
                        m1[64:128, :],
                        rb[64:128, :],
                        op=ALU.add,
                    )

                def emit_vg(n):
                    # v + gates, token-major; the 4 sq-chunks of this n-block
                    # share one psum tile. Gate tanh is one strided ACT op
                    # over all 4 sub-chunks.
                    ps = mixpool.tile([128, 288], FP32, tag="mix", name="psvg")
                    for sub in range(4):
                        s = 4 * n + sub
                        ssl = bass.ts(s, 128)
                        for kk in range(8):
                            nc.tensor.matmul(
                                ps[:, sub * 72 : (sub + 1) * 72],
                                x_sb[kk][:, ssl],
                                wvg_sb[kk][:],
                                start=(sub == 0 and kk == 0),
                                stop=(sub == 3 and kk == 7),
                            )
                    tg = gpool.tile([128, 32], FP32, tag="tg", name="tg")
                    psv = ps[:].rearrange("p (four c) -> p four c", four=4)
                    nc.scalar.activation(
                        tg[:].rearrange("p (four c) -> p four c", four=4),
                        psv[:, :, 64:72],
                        AF.Tanh,
                        scale=0.5,
                    )
                    for sub in range(4):
                        s = 4 * n + sub
                        o = sub * 72
                        nc.vector.tensor_copy(v_sb[s][:, 0:64], ps[:, o : o + 64])
                        nc.vector.memset(v_sb[s][:, 64:65], 1.0)
                        # gate: G = 1 + 0.5*(tanh(a/2) + tanh(b/2))
                        gs = gpool.tile([128, 4], FP32, tag="gs", name="gs")
                        nc.gpsimd.tensor_tensor(
                            gs[:], tg[:, sub * 8 : sub * 8 + 4],
                            tg[:, sub * 8 + 4 : sub * 8 + 8], op=ALU.add
                        )
                        nc.gpsimd.tensor_scalar(
                            g_sb[s][:], gs[:], 0.5, 1.0,
                            op0=ALU.mult, op1=ALU.add,
                        )

                def emit_scores(h, c):
                    """scores.T [kv 128, sq width] -> exp(st/8) bf16 pt tiles
                    (one per 1024-col range)."""
                    qh = qT_sb[h]
                    width = S - c * 128
                    pts = []
                    for t0 in range(0, width, 1024):
                        cols = min(1024, width - t0)
                        st = stpool.tile([128, 1024], FP32, tag="st", name="st")
                        pt = ptpool.tile([128, 1024], BF16, tag="pt", name="pt")
                        pts.append(pt)
                        for n0 in range(0, cols, 512):
                            nn = min(512, cols - n0)
                            nc.tensor.matmul(
                                st[:, n0 : n0 + nn],
                                kT_sb[:, c * 128 : (c + 1) * 128],
                                qh[:, c * 128 + t0 + n0 : c * 128 + t0 + n0 + nn],
                                start=True,
                                stop=True,
                            )
                        nc.scalar.activation(
                            pt[:, 0:cols], st[:, 0:cols], AF.Exp, scale=0.125
                        )
                        if t0 == 0:
                            # causal mask on the diagonal chunk (Pool:
                            # SBUF-only, keeps DVE free)
                            nc.gpsimd.tensor_tensor(
                                pt[:, 0:128], pt[:, 0:128], dmask_sb[:],
                                op=ALU.mult,
                            )
                    return pts

                def emit_sq(h, s, pts_by_c):
                    """P@V over kv chunks for one sq chunk (col 64 = softmax
                    denominator), then the gating epilogue; on head 3 also
                    transpose + output projection + DMA (fp16 partials)."""
                    acc = accpool.tile([128, 65], FP32, tag="acc", name="acc")
                    for c in range(s + 1):
                        off = (s - c) * 128
                        nc.tensor.matmul(
                            acc[:],
                            pts_by_c[c][off // 1024][:, off % 1024 : off % 1024 + 128],
                            v_sb[c][:],
                            start=(c == 0),
                            stop=(c == s),
                        )
                    rl = rlpool.tile([128, 1], FP32, tag="rl", name="rl")
                    nc.vector.reciprocal(rl[:], acc[:, 64:65])
                    nc.vector.tensor_scalar(
                        ag_sb[s][:, h * 64 : (h + 1) * 64],
                        acc[:, 0:64],
                        rl[:],
                        g_sb[s][:, h : h + 1],
                        op0=ALU.mult,
                        op1=ALU.mult,
                    )
                    if h == 3:
                        for j in range(2):
                            tp = mixpool.tile([128, 128], BF16, tag="mix", name="tp")
                            nc.tensor.transpose(
                                tp[:],
                                ag_sb[s][:, j * 128 : (j + 1) * 128],
                                ident_sb[:],
                            )
                            nc.vector.tensor_copy(agT_sb[j][s][:], tp[:])
                        ost = ostpool.tile([128, HM], FP16, tag="ost", name="ost")
                        for nn in range(2):
                            po = mixpool.tile([128, 512], FP32, tag="mix", name="po")
                            for j in range(2):
                                nc.tensor.matmul(
                                    po[:],
                                    agT_sb[j][s][:],
                                    wo_sb[j][:, nn * 512 : (nn + 1) * 512],
                                    start=(j == 0),
                                    stop=(j == 1),
                                )
                            if s >= 10:
                                nc.scalar.copy(
                                    ost[:, nn * 512 : (nn + 1) * 512], po[:]
                                )
                            else:
                                nc.vector.tensor_copy(
                                    ost[:, nn * 512 : (nn + 1) * 512], po[:]
                                )
                        nc.sync.dma_start(outp[s * 128 : (s + 1) * 128, :], ost[:])

                # ---- emission ----
                # Startup: stream x/cos/sin in REVERSE n-block order; after
                # block n arrives, project k/q0/q1 for it and immediately
                # emit head-0 (and head-1 for n>=1) scores for the c-chunks
                # that only need blocks >= n (ascending c within each batch
                # so exp(h,low c) lands first). This keeps ACT fed from ~5us.
                nc.sync.dma_start(wk_all[:], wkT[:, :])
                pts = {h: [None] * NCH for h in range(4)}
                xv = x_all[:].rearrange("p (kk n c) -> p kk n c", kk=8, n=4)
                for n in (3, 2, 1, 0):
                    nc.sync.dma_start(
                        cs_all[:, n * 1024 : (n + 1) * 1024],
                        csT[:, n * 1024 : (n + 1) * 1024],
                    )
                    nc.sync.dma_start(
                        xv[:, :, n, :],
                        xTi[:, n * 4096 : (n + 1) * 4096],
                    )
                    if n == 3:
                        nc.sync.dma_start(wq_all[:], wqT[:, :])
                        nc.sync.dma_start(wvg_all[:], wvgT[:, :])
                        nc.sync.dma_start(dmask_sb[:], dmask[:, :])
                        make_identity(nc, ident_sb[:])
                    if n == 1:
                        for j in range(2):
                            nc.sync.dma_start(
                                wo_sb[j][:], woT[j * 128 : (j + 1) * 128, :]
                            )
                    emit_k_proj(n)
                    emit_q_proj(0, n, qb_on_act=True)
                    for c in range(4 * n, 4 * n + 4):
                        pts[0][c] = emit_scores(0, c)
                    emit_q_proj(1, n, qb_on_act=False)
                    if n >= 1:
                        for c in range(4 * n, 4 * n + 4):
                            pts[1][c] = emit_scores(1, c)
                    emit_vg(n)

                # Phase A: finish h1's wide chunks, then drain h0+h1 P@V
                # (their exps are already queued on ACT).
                for c in range(4):
                    pts[1][c] = emit_scores(1, c)
                for c in range(NCH):
                    emit_sq(0, c, pts[0])
                    emit_sq(1, c, pts[1])

                # Phase B: h2 scores/sq self-interleaved (2-chunk lag); h3
                # scores start at half rate from c>=8 so h3's sq+out-proj
                # spread into this phase instead of piling up in a tail.
                for c in range(NCH):
                    pts[2][c] = emit_scores(2, c)
                    if c >= 2:
                        emit_sq(2, c - 2, pts[2])
                    if c >= 8:
                        pts[3][c - 8] = emit_scores(3, c - 8)
                    if c >= 10:
                        emit_sq(3, c - 10, pts[3])

                # Phase C: finish h3 (sq lags scores by 2), drain.
                emit_sq(2, NCH - 2, pts[2])
                emit_sq(2, NCH - 1, pts[2])
                for c in range(8, NCH):
                    pts[3][c] = emit_scores(3, c)
                    emit_sq(3, c - 2, pts[3])
                emit_sq(3, NCH - 2, pts[3])
                emit_sq(3, NCH - 1, pts[3])

    _split_multi_waits(nc)
    return nc


_NC = None


def _get_nc() -> bass.Bass:
    global _NC
    if _NC is None:
        _NC = _build_program()
    return _NC


def _shard_inputs(
    hidden_states, Wq, Wk, Wv, Wo, Wkc, Wg_slc, Wg_swa
) -> list[dict[str, np.ndarray]]:
    bf16 = ml_dtypes.bfloat16
    f32 = np.float32

    # RoPE tables (bf16, feature-major, duplicated across two 64-row head
    # blocks). The rotation's half-swap is fused into the device multiply's
    # input AP, so the sin table here is stored half-SWAPPED and signed:
    # sinP[d] = +sin for d<32 (pairs with src d+32... device reads
    # sin[src-partition]), -sin for d>=32. Since sin/cos rows repeat with
    # period 32, the swap is a sign flip on the upper half.
    inv = 1.0 / (THETA ** (np.arange(0, D, 2, dtype=np.float64) / D))
    freqs = np.arange(S, dtype=np.float64)[:, None] * inv  # [S, 32]
    cos32 = np.cos(freqs).T  # [32, S]
    sin32 = np.sin(freqs).T
    cosT = np.concatenate([cos32, cos32], axis=0)  # [64, S]
    # device: rb[d<32] = qb[d+32] * sinT[d+32]  (wants -sin)
    #         rb[d>=32] = qb[d-32] * sinT[d-32] (wants +sin)
    sinT = np.concatenate([sin32, -sin32], axis=0)
    cos2 = np.concatenate([cosT, cosT], axis=0).astype(bf16)  # [128, S]
    sin2 = np.concatenate([sinT, sinT], axis=0).astype(bf16)
    # pack per n-block: csT[:, n*1024:(n+1)*1024] = [cos_n | sin_n]
    csT = np.concatenate(
        [
            np.concatenate(
                [cos2[:, n * 512 : (n + 1) * 512], sin2[:, n * 512 : (n + 1) * 512]],
                axis=1,
            )
            for n in range(4)
        ],
        axis=1,
    )
    csT = np.ascontiguousarray(csT)

    # pt[kv_i, sq_j] is valid iff kv <= sq, i.e. i <= j: upper triangular
    dmask = np.triu(np.ones((128, 128), dtype=f32)).astype(bf16)

    def interleave(w):
        """[1024, width] -> [128, 8*width] with hm-chunk-major columns so
        the whole weight loads in one contiguous DMA."""
        width = w.shape[1]
        return np.ascontiguousarray(
            w.reshape(8, 128, width).transpose(1, 0, 2).reshape(128, 8 * width)
        )

    in_maps = []
    for core in range(NCORES):
        b, g = divmod(core, 4)
        # xTi[p, n, kk, j] = x.T[kk*128+p, n*512+j]
        xTc = (
            np.ascontiguousarray(hidden_states[b].T)
            .astype(bf16)
            .reshape(8, 128, 4, 512)
            .transpose(1, 2, 0, 3)
            .reshape(128, 4 * 8 * 512)
        )
        xTc = np.ascontiguousarray(xTc)
        wqTc = interleave(
            np.ascontiguousarray(Wq[g * 256 : (g + 1) * 256, :].T).astype(bf16)
        )
        wkTc = interleave(
            np.ascontiguousarray(Wk[g * 64 : (g + 1) * 64, :].T).astype(bf16)
        )
        wvg = np.concatenate(
            [
                Wv[g * 64 : (g + 1) * 64, :].T,
                Wg_slc[g * 4 : (g + 1) * 4, :].T,
                Wg_swa[g * 4 : (g + 1) * 4, :].T,
            ],
            axis=1,
        )  # [1024, 72]
        wvgc = interleave(np.ascontiguousarray(wvg).astype(bf16))
        woTc = np.ascontiguousarray(Wo[:, g * 256 : (g + 1) * 256].T).astype(bf16)
        in_maps.append(
            {
                "xTi": xTc,
                "wqT": wqTc,
                "wkT": wkTc,
                "wvgT": wvgc,
                "woT": woTc,
                "csT": csT,
                "dmask": dmask,
            }
        )
    return in_maps


def run(inputs: dict, trace: bool = False):
    """Run the SPMD kernel; returns (output [B,S,HM] f32, BassKernelResults)."""
    nc = _get_nc()
    in_maps = _shard_inputs(**inputs)
    res = run_bass_kernel_spmd(
        nc, in_maps, core_ids=list(range(NCORES)), trace=trace
    )
    out = np.zeros((B, S, HM), np.float32)
    for core in range(NCORES):
        b = core // 4
        out[b] += res.results[core]["outp"].astype(np.float32)
    return out, res


def kernel(**inputs) -> np.ndarray:
    out, _ = run(inputs)
    return out
